# revision 54
# baseline (speedup 1.0000x reference)
"""Trainium2 Bass kernel: GPT-2 style causal attention + output projection.

Reference computation (B=2, L=2048, D=1024, H=16, dh=64):
    q,k,v = split_heads(query/key/value)            # [B,H,L,dh]
    S = q @ k^T / sqrt(dh)                          # [B,H,L,L]
    P = softmax(causal_mask(S))
    A = merge_heads(P @ v)                          # [B,L,D]
    out = A @ w_proj + b_proj

Sharding: 32 (b,h) pairs, 4 per core (cores 0-3 batch 0, 4-7 batch 1).
Each core computes attention for its 4 heads fully causally and a partial
c_proj using its 256 rows of w_proj; the host sums the 4 partials per batch
and adds the bias.

Device-side layout: scores are computed transposed (S^T, keys on partitions)
so softmax's P lands with keys on the partition axis, which is exactly the
contraction layout P.V needs.  A ones-column appended to V makes the same
matmul emit softmax denominators.  Two heads ride each 128-partition tile
(dh=64), and their QK matmuls occupy disjoint PE row groups (tile_position)
so they stream concurrently.

The scalar (ACT) engine's exp stream is the pacer (~1us per 128x1024 chunk).
Everything else is scheduled around keeping it and the PE saturated:
  - a slice of the exp work (odd clean chunks; all of J=3, c>=5 elsewhere)
    runs on the vector engine as a Schraudolph-style int16 bit trick
    (round(s*1024*log2e/8 + 15316) bit-viewed as fp16), offloading the
    pacer; validated 1.3e-3 end-to-end on this problem's data.
  - diagonal-band chunks do column-ranged QK/exp/PV (skips fully-masked
    queries), with the causal triangle applied as a 128-col band multiply
    (DVE 4x mode; GpSimd only for the DVE-loaded last block).
  - every matmul operand is fp16: one fp32-mode matmul (e.g. an f32r
    denominator broadcast) makes following fp16 matmuls run 2 cycles/row.
  - chunks are processed in pairs (QK,QK,...,PV,PV) so PE weight loads
    overlap streaming, and PVs trail QKs by 4 chunks.
  - each q-block's c_proj (4 row-tiles x 2x2 duo-major matmuls) and
    normalize chains are deferred into the next block's chunk stream with
    budgeted draining (2 PVs + 1 norm + at most 1 c_proj group per iter).
  - q-block order (2,1,0,3): a long block first covers the ~13us input DMA
    stream; the largest block last has the most exp-stream slack and ends
    in a column-halved normalize+c_proj staircase behind the ranged PVs.
Partials DMA out as fp16 (halves output traffic; host sums in fp32).
"""

import numpy as np

B, L, D, H = 2, 2048, 1024, 16
DH = 64          # head dim
PAIRS = 4        # (b,h) pairs per core
QB = 512         # query block
KC = 128         # key chunk
NCORES = 8

_COMPILED = None


def _build_nc():
    import concourse.bacc as bacc
    import concourse.tile as tile
    from concourse import mybir

    f32 = mybir.dt.float32
    f16 = mybir.dt.float16
    i16 = mybir.dt.int16
    Exp = mybir.ActivationFunctionType.Exp
    Mult = mybir.AluOpType.mult
    Add = mybir.AluOpType.add
    # Schraudolph-style exp-as-bit-trick constants for fp16 output:
    # bits = round(s_raw * 0.125 * 1024/ln2 + (15360 + c)); the int16
    # saturation handles both tails and c=-44 centers the relative error
    # (~+-3%, which washes out in the softmax; validated 1.2e-3 end-to-end)
    SCH_A = 0.125 * 1024.0 / np.log(2.0)
    SCH_B = 15360.0 - 44.0

    nc = bacc.Bacc("TRN2", target_bir_lowering=False, debug=False,
                   num_devices=NCORES)

    qt_d = nc.dram_tensor("qt", [2, 128, L], f16, kind="ExternalInput").ap()
    kt_d = nc.dram_tensor("kt", [2, 128, L], f16, kind="ExternalInput").ap()
    v_d = nc.dram_tensor("v", [PAIRS, 128, (L // KC) * 128], f16,
                         kind="ExternalInput").ap()
    masks_d = nc.dram_tensor("masks", [128, 256], f16, kind="ExternalInput").ap()
    w_d = nc.dram_tensor("w", [2, 128, D], f16, kind="ExternalInput").ap()
    ones_d = nc.dram_tensor("ones", [1, 64], f16, kind="ExternalInput").ap()
    out_d = nc.dram_tensor("out", [L, D], f16, kind="ExternalOutput").ap()

    with tile.TileContext(nc) as tc:
        with (
            tc.tile_pool(name="consts", bufs=1) as consts,
            tc.tile_pool(name="st", bufs=3, space="PSUM") as st_pool,
            tc.tile_pool(name="at", bufs=2, space="PSUM") as at_pool,
            tc.tile_pool(name="et", bufs=10) as et_pool,
            tc.tile_pool(name="atn", bufs=6) as atn_pool,
            tc.tile_pool(name="dsb", bufs=4) as dsb_pool,
            tc.tile_pool(name="rbc", bufs=3) as rbc_pool,
            tc.tile_pool(name="osb", bufs=4) as osb_pool,
        ):
            # resident inputs
            qt = [consts.tile([128, L], f16, name=f"qt{i}", tag=f"qt{i}")
                  for i in range(2)]
            kt = [consts.tile([128, L], f16, name=f"kt{i}", tag=f"kt{i}")
                  for i in range(2)]
            vt = [consts.tile([128, (L // KC) * 128], f16,
                              name=f"vt{i}", tag=f"vt{i}") for i in range(PAIRS)]
            mk = consts.tile([128, 256], f16, name="mk", tag="mk")
            wt = [consts.tile([128, D], f16, name=f"wt{i}", tag=f"wt{i}")
                  for i in range(2)]
            ones = consts.tile([1, 64], f16, name="ones", tag="ones")

            # DMA order follows the block schedule (J=2 first, then 0,1,3):
            # each block's qt/kt slices land just ahead of its stream.
            nc.scalar.dma_start(kt[0][:, 0:512], kt_d[0][:, 0:512])
            nc.sync.dma_start(qt[0][:, 1024:1536], qt_d[0][:, 1024:1536])
            nc.sync.dma_start(kt[0][:, 512:1536], kt_d[0][:, 512:1536])
            nc.sync.dma_start(ones[:], ones_d[:])
            nc.sync.dma_start(vt[0][:], v_d[0])
            nc.sync.dma_start(vt[1][:], v_d[1])
            nc.sync.dma_start(mk[:], masks_d[:])
            nc.sync.dma_start(kt[1][:, 0:1536], kt_d[1][:, 0:1536])
            nc.sync.dma_start(qt[1][:, 1024:1536], qt_d[1][:, 1024:1536])
            nc.sync.dma_start(vt[2][:], v_d[2])
            nc.sync.dma_start(vt[3][:], v_d[3])
            nc.sync.dma_start(qt[0][:, 512:1024], qt_d[0][:, 512:1024])
            nc.sync.dma_start(qt[1][:, 512:1024], qt_d[1][:, 512:1024])
            for i in range(2):
                nc.sync.dma_start(wt[i][:], w_d[i])
            nc.sync.dma_start(qt[0][:, 0:512], qt_d[0][:, 0:512])
            nc.sync.dma_start(qt[1][:, 0:512], qt_d[1][:, 0:512])
            nc.sync.dma_start(qt[0][:, 1536:2048], qt_d[0][:, 1536:2048])
            nc.sync.dma_start(kt[0][:, 1536:2048], kt_d[0][:, 1536:2048])
            nc.sync.dma_start(qt[1][:, 1536:2048], qt_d[1][:, 1536:2048])
            nc.sync.dma_start(kt[1][:, 1536:2048], kt_d[1][:, 1536:2048])

            TAIL = [False]   # after the exp stream ends, ACT is free for obs
            OFFL = [False]   # during DVE-offload regions, ACT takes some obs

            def cproj_groups(J, atn_duo):
                def one(rt):
                    def emit():
                        cp = st_pool.tile([128, D], f32, name="cp", tag="st")
                        # duo-major: consecutive matmuls share lhsT weights
                        for duo in range(2):
                            for nf in range(2):
                                nc.tensor.matmul(
                                    cp[:, nf * 512:(nf + 1) * 512],
                                    lhsT=atn_duo[duo][:, rt * 128:(rt + 1) * 128],
                                    rhs=wt[duo][:, nf * 512:(nf + 1) * 512],
                                    start=(duo == 0), stop=(duo == 1),
                                )
                        ob = osb_pool.tile([128, D], f16, name="ob", tag="ob")
                        if (TAIL[0] or OFFL[0]) and rt % 2:
                            nc.scalar.copy(ob[:], cp[:])
                        else:
                            nc.vector.tensor_copy(ob[:], cp[:])
                        nc.sync.dma_start(
                            out_d[J * QB + rt * 128:J * QB + (rt + 1) * 128, :],
                            ob[:],
                        )
                    return emit
                return [one(rt) for rt in range(QB // 128)]

            pending = []             # c_proj groups from the previous q-block
            pending_norm = []        # softmax-normalize chains
            pending_pv = []          # tail PVs of the previous duo
            for J in (2, 1, 0, 3):   # long block first (covers the input
                                     # DMA stream); the tiny J=0 block sits
                                     # next to slack-rich J=3 so its backlog
                                     # and 2-iteration duos don't squeeze a
                                     # mid-size block's stream
                nch = 4 * J + 4      # causal: key chunks 0..nch-1
                atn_duo = []
                for duo in range(2):
                    OFFL[0] = J in (1, 2, 3)
                    at = [at_pool.tile([128, QB], f32, name="at", tag="at")
                          for _ in range(2)]
                    ets = {}

                    def emit_qk(c):
                        m = c - 4 * J   # diag band index (>=0 on the band)
                        w0 = max(m, 0) * KC   # first causally-live query col
                        st = st_pool.tile([128, 2 * QB], f32, name="st",
                                          tag="st")
                        for h2 in range(2):
                            nc.tensor.matmul(
                                st[:, h2 * QB + w0:(h2 + 1) * QB],
                                lhsT=kt[duo][64 * h2:64 * (h2 + 1),
                                             c * KC:(c + 1) * KC],
                                rhs=qt[duo][64 * h2:64 * (h2 + 1),
                                            J * QB + w0:(J + 1) * QB],
                                start=True, stop=True,
                                tile_position=(64 * h2, 0),
                            )
                        et = et_pool.tile([128, 2 * QB], f16, name="et",
                                          tag="et")
                        # mid-run ACT-paced chunks offload their exp to the
                        # DVE as an int16 bit-trick (the scalar engine is the
                        # pacer there; the DVE has slack)
                        offl = (m < 0 and c % 2 == 1
                                and (J == 3 or (J in (1, 2) and c >= 5)))
                        if offl:
                            nc.vector.tensor_scalar(
                                et[:].bitcast(i16), st[:], SCH_A, SCH_B,
                                Mult, Add)
                        elif w0:
                            nc.scalar.activation(
                                et[:].rearrange("p (h q) -> p h q", h=2)
                                [:, :, w0:],
                                st[:].rearrange("p (h q) -> p h q", h=2)
                                [:, :, w0:],
                                Exp, scale=0.125)
                        else:
                            nc.scalar.activation(et[:], st[:], Exp, scale=0.125)
                        if m >= 0:
                            # causal triangle on the diagonal 128-col band;
                            # columns beyond the band are fully visible, and
                            # columns before it are skipped by the ranged
                            # QK/exp/PV.  DVE's fp16 4x mode is much lower
                            # latency; only the DVE-loaded last block uses
                            # the idle GpSimd engine instead.
                            ev = et[:].rearrange("p (h q) -> p h q", h=2)[
                                :, :, w0:w0 + KC]
                            mv = mk[:].rearrange("p (h q) -> p h q", h=2)
                            eng = nc.gpsimd if J == 3 else nc.vector
                            eng.tensor_mul(ev, ev, mv)
                        ets[c] = et

                    def emit_pv(c, ets=ets, at=at, duo=duo, J=J, nch=nch):
                        m = c - 4 * J
                        w0 = max(m, 0) * KC
                        et = ets.pop(c)
                        for h2 in range(2):
                            pair = 2 * duo + h2
                            nc.tensor.matmul(
                                at[h2][0:128, w0:QB],
                                lhsT=vt[pair][:, c * 128:(c + 1) * 128],
                                rhs=et[:, h2 * QB + w0:(h2 + 1) * QB],
                                start=(c == 0), stop=(c == nch - 1),
                                skip_group_check=True,
                            )

                    def pops(i):
                        # previous-duo tail PVs are latency critical (the
                        # normalize chain and at-slot reuse wait on them);
                        # the norm pops as soon as the PVs are drained.  The
                        # c_proj backlog drains a group per iteration so PE
                        # load stays spread; 16-chunk duos drain faster
                        # (4 PVs at i0, 2 groups/iter) so their diagonal
                        # chunks and the tail staircase get a clean runway.
                        k = 4
                        while k and pending_pv:
                            pending_pv.pop(0)()
                            k -= 1
                        if pending_norm and not pending_pv:
                            pending_norm.pop(0)()
                        cpb = 2 if nch == 16 else 1
                        while cpb and not pending_pv and not pending_norm \
                                and pending:
                            pending.pop(0)()
                            cpb -= 1

                    for i in range(nch // 2):
                        emit_qk(2 * i)
                        emit_qk(2 * i + 1)
                        pops(i)
                        if i >= 2:
                            emit_pv(2 * i - 4)
                            emit_pv(2 * i - 3)
                    # defer the last 4 PVs into the next duo's stream so the
                    # next QKs (and their exps) issue without waiting on this
                    # duo's exp->mask->PV chain
                    pending_pv.extend(
                        [lambda c=c, f=emit_pv: f(c)
                         for c in range(max(nch - 4, 0), nch)])

                    atn = atn_pool.tile([128, QB], f16, name="atn", tag="atn")

                    def norm(at=at, atn=atn):
                        def emit(q0=0, q1=QB):
                            # fp16 dsb/ones keep the PE in fp16 mode (an fp32
                            # matmul halves the following matmuls' row rate)
                            w = q1 - q0
                            dsb = dsb_pool.tile([1, 2 * QB], f16, name="dsb",
                                                tag="dsb")
                            for h2 in range(2):
                                nc.vector.tensor_copy(
                                    dsb[:, h2 * w:(h2 + 1) * w],
                                    at[h2][64:65, q0:q1])
                            bc = st_pool.tile([64, 2 * QB], f32, name="bc",
                                              tag="st")
                            for h2 in range(2):
                                nc.tensor.matmul(
                                    bc[:, h2 * w:(h2 + 1) * w],
                                    lhsT=ones[:],
                                    rhs=dsb[:, h2 * w:(h2 + 1) * w],
                                    start=True, stop=True)
                            rbc = rbc_pool.tile([64, 2 * QB], f32, name="rbc",
                                                tag="rbc")
                            nc.vector.reciprocal_approx_fast(
                                rbc[:, 0:2 * w], bc[:, 0:2 * w])
                            for h2 in range(2):
                                nc.vector.tensor_mul(
                                    atn[64 * h2:64 * (h2 + 1), q0:q1],
                                    at[h2][0:64, q0:q1],
                                    rbc[:, h2 * w:(h2 + 1) * w])
                        return emit

                    if J == 3 and duo == 1:
                        last_norm = norm()   # emitted as a tail staircase
                    else:
                        pending_norm.append(norm())
                    atn_duo.append(atn)

                if J == 3:
                    leftovers = list(pending)   # previous blocks' remainder
                    pending = []
                    final_groups = cproj_groups(3, atn_duo)
                else:
                    # undrained groups stay queued; pops drain them next block
                    pending.extend(cproj_groups(J, atn_duo))
            # tail staircase: at[:, 0:256] of the last duo is final once PVs
            # c0/c1 land (chunk m only touches q >= m*128), so normalize and
            # c_proj proceed in column halves right behind the PVs
            TAIL[0] = True
            OFFL[0] = False
            for g in pending_norm:
                g()
            pending_pv[0]()          # J3 duo1 diag chunks c12/c13
            pending_pv[1]()
            last_norm(0, QB // 2)    # q<256 final: c14/c15 touch q>=256 only
            pending_pv[2]()
            pending_pv[3]()
            for g in final_groups[:2]:
                g()
            last_norm(QB // 2, QB)
            for g in leftovers:
                g()
            for g in final_groups[2:]:
                g()

    nc.compile()
    return nc


def _get_nc():
    global _COMPILED
    if _COMPILED is None:
        _COMPILED = _build_nc()
    return _COMPILED


def _prep_in_maps(query, key, value, w_proj):
    q = np.asarray(query, dtype=np.float32)
    k = np.asarray(key, dtype=np.float32)
    v = np.asarray(value, dtype=np.float32)
    w = np.asarray(w_proj, dtype=np.float32)

    q4 = q.reshape(B, L, H, DH)
    k4 = k.reshape(B, L, H, DH)
    v4 = v.reshape(B, L, H, DH)

    # causal triangle within a diagonal 128x128 block: key row kp visible to
    # query col qf iff kp <= qf; duplicated for the 2 heads per tile
    kp = np.arange(128)[:, None]
    qf = np.arange(128)[None, :]
    tri = (kp <= qf).astype(np.float16)
    masks = np.ascontiguousarray(np.concatenate([tri, tri], axis=1))

    in_maps = []
    for c in range(NCORES):
        b = c // 4
        hsel = 4 * (c % 4)
        qt = np.ascontiguousarray(
            q4[b].transpose(1, 2, 0)[hsel:hsel + 4].reshape(2, 128, L)
            .astype(np.float16))
        kt = np.ascontiguousarray(
            k4[b].transpose(1, 2, 0)[hsel:hsel + 4].reshape(2, 128, L)
            .astype(np.float16))
        vsl = v4[b, :, hsel:hsel + 4, :].transpose(1, 0, 2)  # [4, L, DH]
        vext = np.concatenate(
            [vsl, np.ones((PAIRS, L, 1), dtype=np.float32),
             np.zeros((PAIRS, L, 128 - DH - 1), dtype=np.float32)], axis=2)
        # pre-swizzle to the SBUF layout: [pair, partition, chunk*128]
        vext = (vext.reshape(PAIRS, L // KC, KC, 128)
                .transpose(0, 2, 1, 3).reshape(PAIRS, KC, -1))
        vext = np.ascontiguousarray(vext.astype(np.float16))
        wp = np.ascontiguousarray(
            w[(c % 4) * 256:(c % 4 + 1) * 256, :].reshape(2, 128, D)
            .astype(np.float16))
        in_maps.append({"qt": qt, "kt": kt, "v": vext, "masks": masks,
                        "w": wp, "ones": np.ones((1, 64), dtype=np.float16)})
    return in_maps


def kernel(query, key, value, w_proj, b_proj, n_head):
    from concourse.bass_utils import run_bass_kernel_spmd

    bias = np.asarray(b_proj, dtype=np.float32)
    in_maps = _prep_in_maps(query, key, value, w_proj)
    nc = _get_nc()
    res = run_bass_kernel_spmd(nc, in_maps, list(range(NCORES)))

    out = np.zeros((B, L, D), dtype=np.float32)
    for c in range(NCORES):
        out[c // 4] += res.results[c]["out"].astype(np.float32)
    out += bias[None, None, :]
    return out


# revision 55
# speedup vs baseline: 1.0563x; 1.0563x over previous
"""Trainium2 Bass kernel: GPT-2 style causal attention + output projection.

Reference computation (B=2, L=2048, D=1024, H=16, dh=64):
    q,k,v = split_heads(query/key/value)            # [B,H,L,dh]
    S = q @ k^T / sqrt(dh)                          # [B,H,L,L]
    P = softmax(causal_mask(S))
    A = merge_heads(P @ v)                          # [B,L,D]
    out = A @ w_proj + b_proj

Sharding: 32 (b,h) pairs, 4 per core (cores 0-3 batch 0, 4-7 batch 1).
Each core computes attention for its 4 heads fully causally and a partial
c_proj using its 256 rows of w_proj; the host sums the 4 partials per batch
and adds the bias.

Device-side layout: scores are computed transposed (S^T, keys on partitions)
so softmax's P lands with keys on the partition axis, which is exactly the
contraction layout P.V needs.  A ones-column appended to V makes the same
matmul emit softmax denominators.  Two heads ride each 128-partition tile
(dh=64), and their QK matmuls occupy disjoint PE row groups (tile_position)
so they stream concurrently.

The scalar (ACT) engine's exp stream is the pacer (~1us per 128x1024 chunk).
Everything else is scheduled around keeping it and the PE saturated:
  - a slice of the exp work (odd clean chunks; all of J=3, c>=5 elsewhere)
    runs on the vector engine as a Schraudolph-style int16 bit trick
    (round(s*1024*log2e/8 + 15316) bit-viewed as fp16), offloading the
    pacer; validated 1.3e-3 end-to-end on this problem's data.
  - diagonal-band chunks do column-ranged QK/exp/PV (skips fully-masked
    queries), with the causal triangle applied as a 128-col band multiply
    (DVE 4x mode; GpSimd only for the DVE-loaded last block).
  - every matmul operand is fp16: one fp32-mode matmul (e.g. an f32r
    denominator broadcast) makes following fp16 matmuls run 2 cycles/row.
  - chunks are processed in pairs (QK,QK,...,PV,PV) so PE weight loads
    overlap streaming, and PVs trail QKs by 4 chunks.
  - each q-block's c_proj (4 row-tiles x 2x2 duo-major matmuls) and
    normalize chains are deferred into the next block's chunk stream with
    budgeted draining (2 PVs + 1 norm + at most 1 c_proj group per iter).
  - q-block order (2,1,0,3): a long block first covers the ~13us input DMA
    stream; the largest block last has the most exp-stream slack and ends
    in a column-halved normalize+c_proj staircase behind the ranged PVs.
Partials DMA out as fp16 (halves output traffic; host sums in fp32).
"""

import numpy as np

B, L, D, H = 2, 2048, 1024, 16
DH = 64          # head dim
PAIRS = 4        # (b,h) pairs per core
QB = 512         # query block
KC = 128         # key chunk
NCORES = 8

_COMPILED = None


def _build_nc():
    import concourse.bacc as bacc
    import concourse.tile as tile
    from concourse import mybir

    f32 = mybir.dt.float32
    f16 = mybir.dt.float16
    i16 = mybir.dt.int16
    Exp = mybir.ActivationFunctionType.Exp
    Mult = mybir.AluOpType.mult
    Add = mybir.AluOpType.add
    # Schraudolph-style exp-as-bit-trick constants for fp16 output:
    # bits = round(s_raw * 0.125 * 1024/ln2 + (15360 + c)); the int16
    # saturation handles both tails and c=-44 centers the relative error
    # (~+-3%, which washes out in the softmax; validated 1.2e-3 end-to-end)
    SCH_A = 0.125 * 1024.0 / np.log(2.0)
    SCH_B = 15360.0 - 44.0

    nc = bacc.Bacc("TRN2", target_bir_lowering=False, debug=False,
                   num_devices=NCORES)

    qt_d = nc.dram_tensor("qt", [2, 128, L], f16, kind="ExternalInput").ap()
    kt_d = nc.dram_tensor("kt", [2, 128, L], f16, kind="ExternalInput").ap()
    v_d = nc.dram_tensor("v", [PAIRS, 128, (L // KC) * 128], f16,
                         kind="ExternalInput").ap()
    masks_d = nc.dram_tensor("masks", [128, 256], f16, kind="ExternalInput").ap()
    w_d = nc.dram_tensor("w", [2, 128, D], f16, kind="ExternalInput").ap()
    ones_d = nc.dram_tensor("ones", [1, 64], f16, kind="ExternalInput").ap()
    out_d = nc.dram_tensor("out", [L, D], f16, kind="ExternalOutput").ap()

    with tile.TileContext(nc) as tc:
        with (
            tc.tile_pool(name="consts", bufs=1) as consts,
            tc.tile_pool(name="st", bufs=3, space="PSUM") as st_pool,
            tc.tile_pool(name="at", bufs=2, space="PSUM") as at_pool,
            tc.tile_pool(name="et", bufs=10) as et_pool,
            tc.tile_pool(name="atn", bufs=6) as atn_pool,
            tc.tile_pool(name="dsb", bufs=4) as dsb_pool,
            tc.tile_pool(name="rbc", bufs=3) as rbc_pool,
            tc.tile_pool(name="osb", bufs=4) as osb_pool,
        ):
            # resident inputs
            qt = [consts.tile([128, L], f16, name=f"qt{i}", tag=f"qt{i}")
                  for i in range(2)]
            kt = [consts.tile([128, L], f16, name=f"kt{i}", tag=f"kt{i}")
                  for i in range(2)]
            vt = [consts.tile([128, (L // KC) * 128], f16,
                              name=f"vt{i}", tag=f"vt{i}") for i in range(PAIRS)]
            mk = consts.tile([128, 256], f16, name="mk", tag="mk")
            wt = [consts.tile([128, D], f16, name=f"wt{i}", tag=f"wt{i}")
                  for i in range(2)]
            ones = consts.tile([1, 64], f16, name="ones", tag="ones")

            # DMA order follows the block schedule (J=2 first, then 0,1,3):
            # each block's qt/kt slices land just ahead of its stream.
            nc.scalar.dma_start(kt[0][:, 0:512], kt_d[0][:, 0:512])
            nc.sync.dma_start(qt[0][:, 1024:1536], qt_d[0][:, 1024:1536])
            nc.sync.dma_start(kt[0][:, 512:1536], kt_d[0][:, 512:1536])
            nc.sync.dma_start(ones[:], ones_d[:])
            nc.sync.dma_start(vt[0][:], v_d[0])
            nc.sync.dma_start(vt[1][:], v_d[1])
            nc.sync.dma_start(mk[:], masks_d[:])
            nc.sync.dma_start(kt[1][:, 0:1536], kt_d[1][:, 0:1536])
            nc.sync.dma_start(qt[1][:, 1024:1536], qt_d[1][:, 1024:1536])
            nc.sync.dma_start(vt[2][:], v_d[2])
            nc.sync.dma_start(vt[3][:], v_d[3])
            nc.sync.dma_start(qt[0][:, 512:1024], qt_d[0][:, 512:1024])
            nc.sync.dma_start(qt[1][:, 512:1024], qt_d[1][:, 512:1024])
            for i in range(2):
                nc.sync.dma_start(wt[i][:], w_d[i])
            nc.sync.dma_start(qt[0][:, 0:512], qt_d[0][:, 0:512])
            nc.sync.dma_start(qt[1][:, 0:512], qt_d[1][:, 0:512])
            nc.sync.dma_start(qt[0][:, 1536:2048], qt_d[0][:, 1536:2048])
            nc.sync.dma_start(kt[0][:, 1536:2048], kt_d[0][:, 1536:2048])
            nc.sync.dma_start(qt[1][:, 1536:2048], qt_d[1][:, 1536:2048])
            nc.sync.dma_start(kt[1][:, 1536:2048], kt_d[1][:, 1536:2048])

            TAIL = [False]   # after the exp stream ends, ACT is free for obs
            OFFL = [False]   # during DVE-offload regions, ACT takes some obs

            def cproj_groups(J, atn_duo):
                def one(rt):
                    def emit():
                        cp = st_pool.tile([128, D], f32, name="cp", tag="st")
                        # duo-major: consecutive matmuls share lhsT weights
                        for duo in range(2):
                            for nf in range(2):
                                nc.tensor.matmul(
                                    cp[:, nf * 512:(nf + 1) * 512],
                                    lhsT=atn_duo[duo][:, rt * 128:(rt + 1) * 128],
                                    rhs=wt[duo][:, nf * 512:(nf + 1) * 512],
                                    start=(duo == 0), stop=(duo == 1),
                                )
                        ob = osb_pool.tile([128, D], f16, name="ob", tag="ob")
                        if (TAIL[0] or OFFL[0]) and rt % 2:
                            nc.scalar.copy(ob[:], cp[:])
                        else:
                            nc.vector.tensor_copy(ob[:], cp[:])
                        nc.sync.dma_start(
                            out_d[J * QB + rt * 128:J * QB + (rt + 1) * 128, :],
                            ob[:],
                        )
                    return emit
                return [one(rt) for rt in range(QB // 128)]

            pending = []             # c_proj groups from the previous q-block
            pending_norm = []        # softmax-normalize chains
            pending_pv = []          # tail PVs of the previous duo
            for J in (2, 1, 0, 3):   # long block first (covers the input
                                     # DMA stream); the tiny J=0 block sits
                                     # next to slack-rich J=3 so its backlog
                                     # and 2-iteration duos don't squeeze a
                                     # mid-size block's stream
                nch = 4 * J + 4      # causal: key chunks 0..nch-1
                atn_duo = []
                for duo in range(2):
                    OFFL[0] = J in (1, 2, 3)
                    at = [at_pool.tile([128, QB], f32, name="at", tag="at")
                          for _ in range(2)]
                    ets = {}

                    def emit_qk(c):
                        m = c - 4 * J   # diag band index (>=0 on the band)
                        w0 = max(m, 0) * KC   # first causally-live query col
                        st = st_pool.tile([128, 2 * QB], f32, name="st",
                                          tag="st")
                        for h2 in range(2):
                            nc.tensor.matmul(
                                st[:, h2 * QB + w0:(h2 + 1) * QB],
                                lhsT=kt[duo][64 * h2:64 * (h2 + 1),
                                             c * KC:(c + 1) * KC],
                                rhs=qt[duo][64 * h2:64 * (h2 + 1),
                                            J * QB + w0:(J + 1) * QB],
                                start=True, stop=True,
                                tile_position=(64 * h2, 0),
                            )
                        et = et_pool.tile([128, 2 * QB], f16, name="et",
                                          tag="et")
                        # mid-run ACT-paced chunks offload their exp to the
                        # DVE as an int16 bit-trick (the scalar engine is the
                        # pacer there; the DVE has slack)
                        offl = (m < 0 and c % 2 == 1
                                and (J == 3 or (J in (1, 2) and c >= 5)))
                        if offl:
                            nc.vector.tensor_scalar(
                                et[:].bitcast(i16), st[:], SCH_A, SCH_B,
                                Mult, Add)
                        elif w0:
                            nc.scalar.activation(
                                et[:].rearrange("p (h q) -> p h q", h=2)
                                [:, :, w0:],
                                st[:].rearrange("p (h q) -> p h q", h=2)
                                [:, :, w0:],
                                Exp, scale=0.125)
                        else:
                            nc.scalar.activation(et[:], st[:], Exp, scale=0.125)
                        if m >= 0:
                            # causal triangle on the diagonal 128-col band;
                            # columns beyond the band are fully visible, and
                            # columns before it are skipped by the ranged
                            # QK/exp/PV.  DVE's fp16 4x mode is much lower
                            # latency; only the DVE-loaded last block uses
                            # the idle GpSimd engine instead.
                            ev = et[:].rearrange("p (h q) -> p h q", h=2)[
                                :, :, w0:w0 + KC]
                            mv = mk[:].rearrange("p (h q) -> p h q", h=2)
                            eng = nc.gpsimd if J == 3 else nc.vector
                            eng.tensor_mul(ev, ev, mv)
                        ets[c] = et

                    def emit_pv(c, ets=ets, at=at, duo=duo, J=J, nch=nch):
                        m = c - 4 * J
                        w0 = max(m, 0) * KC
                        et = ets.pop(c)
                        for h2 in range(2):
                            pair = 2 * duo + h2
                            nc.tensor.matmul(
                                at[h2][0:128, w0:QB],
                                lhsT=vt[pair][:, c * 128:(c + 1) * 128],
                                rhs=et[:, h2 * QB + w0:(h2 + 1) * QB],
                                start=(c == 0), stop=(c == nch - 1),
                                skip_group_check=True,
                            )

                    def pops(i):
                        # previous-duo tail PVs are latency critical (the
                        # normalize chain and at-slot reuse wait on them);
                        # the norm pops as soon as the PVs are drained.  The
                        # c_proj backlog drains a group per iteration so PE
                        # load stays spread; 16-chunk duos drain faster
                        # (4 PVs at i0, 2 groups/iter) so their diagonal
                        # chunks and the tail staircase get a clean runway.
                        k = 4
                        while k and pending_pv:
                            pending_pv.pop(0)()
                            k -= 1
                        if pending_norm and not pending_pv:
                            pending_norm.pop(0)()
                        cpb = 2 if nch == 16 else 1
                        while cpb and i >= 1 and not pending_pv \
                                and not pending_norm and pending:
                            pending.pop(0)()
                            cpb -= 1

                    for i in range(nch // 2):
                        emit_qk(2 * i)
                        emit_qk(2 * i + 1)
                        pops(i)
                        if i >= 2:
                            emit_pv(2 * i - 4)
                            emit_pv(2 * i - 3)
                    # defer the last 4 PVs into the next duo's stream so the
                    # next QKs (and their exps) issue without waiting on this
                    # duo's exp->mask->PV chain
                    pending_pv.extend(
                        [lambda c=c, f=emit_pv: f(c)
                         for c in range(max(nch - 4, 0), nch)])

                    atn = atn_pool.tile([128, QB], f16, name="atn", tag="atn")

                    def norm(at=at, atn=atn):
                        def emit(q0=0, q1=QB):
                            # fp16 dsb/ones keep the PE in fp16 mode (an fp32
                            # matmul halves the following matmuls' row rate)
                            w = q1 - q0
                            dsb = dsb_pool.tile([1, 2 * QB], f16, name="dsb",
                                                tag="dsb")
                            for h2 in range(2):
                                nc.vector.tensor_copy(
                                    dsb[:, h2 * w:(h2 + 1) * w],
                                    at[h2][64:65, q0:q1])
                            bc = st_pool.tile([64, 2 * QB], f32, name="bc",
                                              tag="st")
                            for h2 in range(2):
                                nc.tensor.matmul(
                                    bc[:, h2 * w:(h2 + 1) * w],
                                    lhsT=ones[:],
                                    rhs=dsb[:, h2 * w:(h2 + 1) * w],
                                    start=True, stop=True)
                            rbc = rbc_pool.tile([64, 2 * QB], f32, name="rbc",
                                                tag="rbc")
                            nc.vector.reciprocal_approx_fast(
                                rbc[:, 0:2 * w], bc[:, 0:2 * w])
                            for h2 in range(2):
                                nc.vector.tensor_mul(
                                    atn[64 * h2:64 * (h2 + 1), q0:q1],
                                    at[h2][0:64, q0:q1],
                                    rbc[:, h2 * w:(h2 + 1) * w])
                        return emit

                    if J == 3 and duo == 1:
                        last_norm = norm()   # emitted as a tail staircase
                    else:
                        pending_norm.append(norm())
                    atn_duo.append(atn)

                if J == 3:
                    leftovers = list(pending)   # previous blocks' remainder
                    pending = []
                    final_groups = cproj_groups(3, atn_duo)
                else:
                    # undrained groups stay queued; pops drain them next block
                    pending.extend(cproj_groups(J, atn_duo))
            # tail staircase: at[:, 0:256] of the last duo is final once PVs
            # c0/c1 land (chunk m only touches q >= m*128), so normalize and
            # c_proj proceed in column halves right behind the PVs
            TAIL[0] = True
            OFFL[0] = False
            for g in pending_norm:
                g()
            pending_pv[0]()          # J3 duo1 diag chunks c12/c13
            pending_pv[1]()
            last_norm(0, QB // 2)    # q<256 final: c14/c15 touch q>=256 only
            pending_pv[2]()
            pending_pv[3]()
            for g in final_groups[:2]:
                g()
            last_norm(QB // 2, QB)
            for g in leftovers:
                g()
            for g in final_groups[2:]:
                g()

    nc.compile()
    return nc


def _get_nc():
    global _COMPILED
    if _COMPILED is None:
        _COMPILED = _build_nc()
    return _COMPILED


def _prep_in_maps(query, key, value, w_proj):
    q = np.asarray(query, dtype=np.float32)
    k = np.asarray(key, dtype=np.float32)
    v = np.asarray(value, dtype=np.float32)
    w = np.asarray(w_proj, dtype=np.float32)

    q4 = q.reshape(B, L, H, DH)
    k4 = k.reshape(B, L, H, DH)
    v4 = v.reshape(B, L, H, DH)

    # causal triangle within a diagonal 128x128 block: key row kp visible to
    # query col qf iff kp <= qf; duplicated for the 2 heads per tile
    kp = np.arange(128)[:, None]
    qf = np.arange(128)[None, :]
    tri = (kp <= qf).astype(np.float16)
    masks = np.ascontiguousarray(np.concatenate([tri, tri], axis=1))

    in_maps = []
    for c in range(NCORES):
        b = c // 4
        hsel = 4 * (c % 4)
        qt = np.ascontiguousarray(
            q4[b].transpose(1, 2, 0)[hsel:hsel + 4].reshape(2, 128, L)
            .astype(np.float16))
        kt = np.ascontiguousarray(
            k4[b].transpose(1, 2, 0)[hsel:hsel + 4].reshape(2, 128, L)
            .astype(np.float16))
        vsl = v4[b, :, hsel:hsel + 4, :].transpose(1, 0, 2)  # [4, L, DH]
        vext = np.concatenate(
            [vsl, np.ones((PAIRS, L, 1), dtype=np.float32),
             np.zeros((PAIRS, L, 128 - DH - 1), dtype=np.float32)], axis=2)
        # pre-swizzle to the SBUF layout: [pair, partition, chunk*128]
        vext = (vext.reshape(PAIRS, L // KC, KC, 128)
                .transpose(0, 2, 1, 3).reshape(PAIRS, KC, -1))
        vext = np.ascontiguousarray(vext.astype(np.float16))
        wp = np.ascontiguousarray(
            w[(c % 4) * 256:(c % 4 + 1) * 256, :].reshape(2, 128, D)
            .astype(np.float16))
        in_maps.append({"qt": qt, "kt": kt, "v": vext, "masks": masks,
                        "w": wp, "ones": np.ones((1, 64), dtype=np.float16)})
    return in_maps


def kernel(query, key, value, w_proj, b_proj, n_head):
    from concourse.bass_utils import run_bass_kernel_spmd

    bias = np.asarray(b_proj, dtype=np.float32)
    in_maps = _prep_in_maps(query, key, value, w_proj)
    nc = _get_nc()
    res = run_bass_kernel_spmd(nc, in_maps, list(range(NCORES)))

    out = np.zeros((B, L, D), dtype=np.float32)
    for c in range(NCORES):
        out[c // 4] += res.results[c]["out"].astype(np.float32)
    out += bias[None, None, :]
    return out
